# revision 6
# baseline (speedup 1.0000x reference)
"""TRN2 Bass kernel for nn_Inv_Attn (SAGAN-style self-attention block).

Math per batch b (C=512, C2=256, N=2304):
  F = Wf @ w + bf          (C2, N)   [from input w]
  G = Wg @ x + bg          (C2, N)
  HxT[j,o] = (Wh @ x + bh).T          (N, C)
  A'[j,i] = sum_o G[o,j] F[o,i]       (N, N)   == attn_dist[i,j]
  T[j,:]  = softmax(A'[j,:] - 64)     (row softmax; fixed shift, no max pass)
  out2[b] = T
  out1[b,c,i] = gamma * sum_j HxT[j,c] T[j,i] + w[b,c,i]

Sharding: data-parallel over batch, 2 batches per core across 8 cores.
Precision: fp32r (PE high-portion fp32) for F/G/A'/HxT matmuls; bf16 for
the big out matmul and for stored attention weights (out2 returned fp32
after host upcast).
"""

import sys

sys.path.insert(0, "/opt/trn_rl_repo")

import numpy as np
import ml_dtypes

import concourse.bass as bass
from concourse import bacc
from concourse import mybir
from concourse.tile import TileContext
from concourse.bass_utils import run_bass_kernel_spmd

B, C, H, W_ = 16, 512, 48, 48
N = H * W_  # 2304
C2 = C // 2  # 256
NB = 2  # batches per core
NCORES = 8
KC = 4  # C / 128
OC = 2  # C2 / 128
NJB = N // 128  # 18 j row-blocks
JH = NJB // 2  # 9 per half
ISL = [(0, 512), (512, 512), (1024, 512), (1536, 512), (2048, 256)]
CCH = 4  # C / 128 output channel chunks

F32 = mybir.dt.float32
F32R = mybir.dt.float32r
BF16 = mybir.dt.bfloat16
EXP_SHIFT = -64.0


def build(gamma: float):
    nc = bacc.Bacc(num_swdge_queues=4)
    x = nc.dram_tensor("x", [NB, C, N], F32, kind="ExternalInput")
    w = nc.dram_tensor("w", [NB, C, N], F32, kind="ExternalInput")
    wfT = nc.dram_tensor("wfT", [C, C2], F32, kind="ExternalInput")
    wgT = nc.dram_tensor("wgT", [C, C2], F32, kind="ExternalInput")
    whT = nc.dram_tensor("whT", [C, C], F32, kind="ExternalInput")
    bf = nc.dram_tensor("bf", [C2, 1], F32, kind="ExternalInput")
    bg = nc.dram_tensor("bg", [C2, 1], F32, kind="ExternalInput")
    bh = nc.dram_tensor("bh", [1, C], F32, kind="ExternalInput")
    out1 = nc.dram_tensor("out1", [NB, C, N], F32, kind="ExternalOutput")
    out2 = nc.dram_tensor("out2", [NB, N, N], BF16, kind="ExternalOutput")

    with TileContext(nc) as tc:
        with (
            tc.tile_pool(name="wts", bufs=1) as wts,
            tc.tile_pool(name="xw", bufs=4) as xw,
            tc.tile_pool(name="stg", bufs=2) as stg,
            tc.tile_pool(name="fg", bufs=1) as fg,
            tc.tile_pool(name="hx", bufs=1) as hx,
            tc.tile_pool(name="ebp", bufs=9) as ebp,
            tc.tile_pool(name="accp", bufs=1) as accp,
            tc.tile_pool(name="smalls", bufs=4) as smalls,
            tc.tile_pool(name="outp", bufs=3) as outp,
            tc.tile_pool(name="ps", bufs=1, space="PSUM") as ps,
        ):
            # ---- persistent weights (fp32r in SBUF via gpsimd cast DMA) ----
            wfT_t = []
            wgT_t = []
            whT_t = []
            for kc in range(KC):
                tf = wts.tile([128, C2], F32R, name=f"wfT{kc}", tag=f"wfT{kc}")
                nc.gpsimd.dma_start(out=tf, in_=wfT[kc * 128 : (kc + 1) * 128, :])
                wfT_t.append(tf)
                tg = wts.tile([128, C2], F32R, name=f"wgT{kc}", tag=f"wgT{kc}")
                nc.gpsimd.dma_start(out=tg, in_=wgT[kc * 128 : (kc + 1) * 128, :])
                wgT_t.append(tg)
                th = wts.tile([128, C], F32R, name=f"whT{kc}", tag=f"whT{kc}")
                nc.gpsimd.dma_start(out=th, in_=whT[kc * 128 : (kc + 1) * 128, :])
                whT_t.append(th)
            bf_t = []
            bg_t = []
            for oc in range(OC):
                t1 = wts.tile([128, 1], F32, name=f"bf_t{oc}", tag=f"bf_t{oc}")
                nc.sync.dma_start(out=t1, in_=bf[oc * 128 : (oc + 1) * 128, :])
                bf_t.append(t1)
                t2 = wts.tile([128, 1], F32, name=f"bg_t{oc}", tag=f"bg_t{oc}")
                nc.sync.dma_start(out=t2, in_=bg[oc * 128 : (oc + 1) * 128, :])
                bg_t.append(t2)
            bh_t = wts.tile([1, C], BF16, name="bh_t", tag="bh_t")
            nc.gpsimd.dma_start(out=bh_t, in_=bh[:, :])
            ones_t = wts.tile([1, 128], BF16, name="ones_t", tag="ones_t")
            nc.vector.memset(ones_t, 1.0)
            shift_t = wts.tile([128, 1], F32, name="shift_t", tag="shift_t")
            nc.vector.memset(shift_t, EXP_SHIFT)

            for b in range(NB):
                # ================= phase A =================
                # W input -> F (then X reuses the same slots -> G, HxT)
                w_t = []
                for kc in range(KC):
                    wt = xw.tile([128, N], F32R, name=f"w_{b}_{kc}", tag="xw")
                    for hh in range(2):
                        st = stg.tile([128, N // 2], F32, name="stg_t", tag="stg")
                        nc.sync.dma_start(
                            out=st,
                            in_=w[
                                b,
                                kc * 128 : (kc + 1) * 128,
                                hh * (N // 2) : (hh + 1) * (N // 2),
                            ],
                        )
                        nc.gpsimd.tensor_copy(
                            wt[:, hh * (N // 2) : (hh + 1) * (N // 2)], st
                        )
                    w_t.append(wt)
                F_t = [
                    fg.tile([128, N], F32R, name=f"F{b}{oc}", tag=f"F{oc}")
                    for oc in range(OC)
                ]
                G_t = [
                    fg.tile([128, N], F32R, name=f"G{b}{oc}", tag=f"G{oc}")
                    for oc in range(OC)
                ]
                for oc in range(OC):
                    for i0, iw in ISL:
                        p = ps.tile([128, 512], F32, name="psA", tag="out", bufs=5)
                        for kc in range(KC):
                            nc.tensor.matmul(
                                p[:, :iw],
                                wfT_t[kc][:, oc * 128 : (oc + 1) * 128],
                                w_t[kc][:, i0 : i0 + iw],
                                start=(kc == 0),
                                stop=(kc == KC - 1),
                            )
                        nc.scalar.activation(
                            F_t[oc][:, i0 : i0 + iw],
                            p[:, :iw],
                            mybir.ActivationFunctionType.Identity,
                            bias=bf_t[oc],
                            scale=1.0,
                        )
                x_t = []
                for kc in range(KC):
                    xt = xw.tile([128, N], F32R, name=f"x_{b}_{kc}", tag="xw")
                    for hh in range(2):
                        st = stg.tile([128, N // 2], F32, name="stg_t", tag="stg")
                        nc.sync.dma_start(
                            out=st,
                            in_=x[
                                b,
                                kc * 128 : (kc + 1) * 128,
                                hh * (N // 2) : (hh + 1) * (N // 2),
                            ],
                        )
                        nc.gpsimd.tensor_copy(
                            xt[:, hh * (N // 2) : (hh + 1) * (N // 2)], st
                        )
                    x_t.append(xt)
                for oc in range(OC):
                    for i0, iw in ISL:
                        p = ps.tile([128, 512], F32, name="psG", tag="out", bufs=5)
                        for kc in range(KC):
                            nc.tensor.matmul(
                                p[:, :iw],
                                wgT_t[kc][:, oc * 128 : (oc + 1) * 128],
                                x_t[kc][:, i0 : i0 + iw],
                                start=(kc == 0),
                                stop=(kc == KC - 1),
                            )
                        nc.scalar.activation(
                            G_t[oc][:, i0 : i0 + iw],
                            p[:, :iw],
                            mybir.ActivationFunctionType.Identity,
                            bias=bg_t[oc],
                            scale=1.0,
                        )
                hxTb = []
                for jb in range(NJB):
                    p = ps.tile([128, 512], F32, name="psH", tag="out", bufs=5)
                    for kc in range(KC):
                        nc.tensor.matmul(
                            p,
                            x_t[kc][:, jb * 128 : (jb + 1) * 128],
                            whT_t[kc],
                            start=(kc == 0),
                            stop=False,
                        )
                    nc.tensor.matmul(p, ones_t, bh_t, start=False, stop=True)
                    ht = hx.tile(
                        [128, C], BF16, name=f"hxT{b}_{jb}", tag=f"hxT{jb}"
                    )
                    nc.scalar.copy(ht, p)
                    hxTb.append(ht)

                # ======== phases B/C in two j-halves ========
                acc_t = [
                    accp.tile([128, N], BF16, name=f"acc{b}{cc}", tag=f"acc{cc}")
                    for cc in range(CCH)
                ]
                for half in range(2):
                    jbs = range(half * JH, (half + 1) * JH)
                    eb_half = []
                    # ---- phase B: attention rows + softmax ----
                    for jb in jbs:
                        eb_t = ebp.tile([128, N], BF16, name=f"eb{jb}", tag="eb")
                        part = smalls.tile(
                            [128, 8], F32, name=f"part{jb}", tag="part"
                        )
                        for it, (i0, iw) in enumerate(ISL):
                            p = ps.tile(
                                [128, 512], F32, name="psAt", tag="attn", bufs=3
                            )
                            for oc in range(OC):
                                nc.tensor.matmul(
                                    p[:, :iw],
                                    G_t[oc][:, jb * 128 : (jb + 1) * 128],
                                    F_t[oc][:, i0 : i0 + iw],
                                    start=(oc == 0),
                                    stop=(oc == OC - 1),
                                )
                            nc.scalar.activation(
                                eb_t[:, i0 : i0 + iw],
                                p[:, :iw],
                                mybir.ActivationFunctionType.Exp,
                                bias=shift_t,
                                scale=1.0,
                                accum_out=part[:, it : it + 1],
                            )
                        rs = smalls.tile([128, 1], F32, name=f"rs{jb}", tag="rs")
                        nc.vector.reduce_sum(
                            out=rs, in_=part[:, 0:5], axis=mybir.AxisListType.X
                        )
                        rinv = smalls.tile(
                            [128, 1], F32, name=f"rinv{jb}", tag="rinv"
                        )
                        nc.vector.reciprocal(rinv, rs)
                        nc.vector.tensor_scalar_mul(eb_t, eb_t, rinv)
                        nc.sync.dma_start(
                            out=out2[b, jb * 128 : (jb + 1) * 128, :], in_=eb_t
                        )
                        eb_half.append(eb_t)
                    # ---- phase C: out += HxT.T @ Eb for this half ----
                    for i0, iw in ISL:
                        for cc in range(CCH):
                            p = ps.tile(
                                [128, 512], F32, name="psO", tag="out", bufs=5
                            )
                            for k, jb in enumerate(jbs):
                                nc.tensor.matmul(
                                    p[:, :iw],
                                    hxTb[jb][:, cc * 128 : (cc + 1) * 128],
                                    eb_half[k][:, i0 : i0 + iw],
                                    start=(k == 0),
                                    stop=(k == JH - 1),
                                )
                            if half == 0:
                                nc.scalar.copy(
                                    acc_t[cc][:, i0 : i0 + iw], p[:, :iw]
                                )
                            else:
                                e = outp.tile(
                                    [128, 512], F32, name="e_t", tag="e_t", bufs=2
                                )
                                nc.vector.tensor_add(
                                    e[:, :iw], p[:, :iw], acc_t[cc][:, i0 : i0 + iw]
                                )
                                wct = outp.tile(
                                    [128, 512], F32, name="wct", tag="wct", bufs=2
                                )
                                nc.sync.dma_start(
                                    out=wct[:, :iw],
                                    in_=w[b, cc * 128 : (cc + 1) * 128, i0 : i0 + iw],
                                )
                                o = outp.tile(
                                    [128, 512], F32, name="o_t", tag="o_t", bufs=2
                                )
                                nc.scalar.mul(e[:, :iw], e[:, :iw], gamma)
                                nc.vector.tensor_add(o[:, :iw], e[:, :iw], wct[:, :iw])
                                nc.sync.dma_start(
                                    out=out1[
                                        b, cc * 128 : (cc + 1) * 128, i0 : i0 + iw
                                    ],
                                    in_=o[:, :iw],
                                )
    nc.finalize()
    return nc


_CACHE = {}


def kernel(x, w, Wf, bf, Wg, bg, Wh, bh, gamma):
    x = np.asarray(x, dtype=np.float32).reshape(B, C, N)
    w = np.asarray(w, dtype=np.float32).reshape(B, C, N)
    g = float(np.asarray(gamma))
    key = ("k", g)
    if key not in _CACHE:
        _CACHE[key] = build(g)
    nc = _CACHE[key]

    wfT = np.ascontiguousarray(np.asarray(Wf, np.float32).T)
    wgT = np.ascontiguousarray(np.asarray(Wg, np.float32).T)
    whT = np.ascontiguousarray(np.asarray(Wh, np.float32).T)
    bf2 = np.ascontiguousarray(np.asarray(bf, np.float32).reshape(C2, 1))
    bg2 = np.ascontiguousarray(np.asarray(bg, np.float32).reshape(C2, 1))
    bh2 = np.ascontiguousarray(np.asarray(bh, np.float32).reshape(1, C))

    in_maps = []
    for c in range(NCORES):
        in_maps.append(
            {
                "x": np.ascontiguousarray(x[c * NB : (c + 1) * NB]),
                "w": np.ascontiguousarray(w[c * NB : (c + 1) * NB]),
                "wfT": wfT,
                "wgT": wgT,
                "whT": whT,
                "bf": bf2,
                "bg": bg2,
                "bh": bh2,
            }
        )
    res = run_bass_kernel_spmd(nc, in_maps, core_ids=list(range(NCORES)))
    o1 = np.concatenate([r["out1"] for r in res.results], axis=0)
    o2 = np.concatenate(
        [r["out2"].astype(np.float32) for r in res.results], axis=0
    )
    return o1.reshape(B, C, H, W_), o2


# revision 7
# speedup vs baseline: 1.5919x; 1.5919x over previous
"""TRN2 Bass kernel for nn_Inv_Attn (SAGAN-style self-attention block).

Math per batch b (C=512, C2=256, N=2304):
  F = Wf @ w + bf          (C2, N)   [from input w]
  G = Wg @ x + bg          (C2, N)
  HxT[j,o] = (Wh @ x + bh).T          (N, C)
  A'[j,i] = sum_o G[o,j] F[o,i]       (N, N)   == attn_dist[i,j]
  T[j,:]  = softmax(A'[j,:] - 64)     (row softmax; fixed shift, no max pass)
  out2[b] = T
  out1[b,c,i] = gamma * sum_j HxT[j,c] T[j,i] + w[b,c,i]

Sharding: data-parallel over batch, 2 batches per core across 8 cores.
Precision: fp32r (PE high-portion fp32) for F/G/A'/HxT matmuls; bf16 for
the big out matmul and for stored attention weights (out2 returned fp32
after host upcast).
"""

import sys

sys.path.insert(0, "/opt/trn_rl_repo")

import numpy as np
import ml_dtypes

import concourse.bass as bass
from concourse import bacc
from concourse import mybir
from concourse.tile import TileContext
from concourse.bass_utils import run_bass_kernel_spmd

B, C, H, W_ = 16, 512, 48, 48
N = H * W_  # 2304
C2 = C // 2  # 256
NB = 2  # batches per core
NCORES = 8
KC = 4  # C / 128
OC = 2  # C2 / 128
NJB = N // 128  # 18 j row-blocks
JH = NJB // 2  # 9 per half
ISL = [(0, 512), (512, 512), (1024, 512), (1536, 512), (2048, 256)]
CCH = 4  # C / 128 output channel chunks

F32 = mybir.dt.float32
F32R = mybir.dt.float32r
BF16 = mybir.dt.bfloat16
EXP_SHIFT = -64.0


def build(gamma: float):
    nc = bacc.Bacc(num_swdge_queues=4)
    x = nc.dram_tensor("x", [NB, C, N], F32, kind="ExternalInput")
    w = nc.dram_tensor("w", [NB, C, N], F32, kind="ExternalInput")
    wfT = nc.dram_tensor("wfT", [C, C2], F32, kind="ExternalInput")
    wgT = nc.dram_tensor("wgT", [C, C2], F32, kind="ExternalInput")
    whT = nc.dram_tensor("whT", [C, C], F32, kind="ExternalInput")
    bf = nc.dram_tensor("bf", [C2, 1], F32, kind="ExternalInput")
    bg = nc.dram_tensor("bg", [C2, 1], F32, kind="ExternalInput")
    bh = nc.dram_tensor("bh", [1, C], F32, kind="ExternalInput")
    out1 = nc.dram_tensor("out1", [NB, C, N], F32, kind="ExternalOutput")
    out2 = nc.dram_tensor("out2", [NB, N, N], BF16, kind="ExternalOutput")

    with TileContext(nc) as tc:
        with (
            tc.tile_pool(name="wts", bufs=1) as wts,
            tc.tile_pool(name="xw", bufs=4) as xw,
            tc.tile_pool(name="stg", bufs=2) as stg,
            tc.tile_pool(name="fg", bufs=1) as fg,
            tc.tile_pool(name="hx", bufs=1) as hx,
            tc.tile_pool(name="ebp", bufs=9) as ebp,
            tc.tile_pool(name="accp", bufs=1) as accp,
            tc.tile_pool(name="smalls", bufs=4) as smalls,
            tc.tile_pool(name="outp", bufs=3) as outp,
            tc.tile_pool(name="ps", bufs=1, space="PSUM") as ps,
        ):
            # ---- persistent weights (fp32r in SBUF via gpsimd cast DMA) ----
            wfT_t = []
            wgT_t = []
            whT_t = []
            for kc in range(KC):
                tf = wts.tile([128, C2], F32R, name=f"wfT{kc}", tag=f"wfT{kc}")
                nc.gpsimd.dma_start(out=tf, in_=wfT[kc * 128 : (kc + 1) * 128, :])
                wfT_t.append(tf)
                tg = wts.tile([128, C2], F32R, name=f"wgT{kc}", tag=f"wgT{kc}")
                nc.gpsimd.dma_start(out=tg, in_=wgT[kc * 128 : (kc + 1) * 128, :])
                wgT_t.append(tg)
                th = wts.tile([128, C], F32R, name=f"whT{kc}", tag=f"whT{kc}")
                nc.gpsimd.dma_start(out=th, in_=whT[kc * 128 : (kc + 1) * 128, :])
                whT_t.append(th)
            bf_t = []
            bg_t = []
            for oc in range(OC):
                t1 = wts.tile([128, 1], F32, name=f"bf_t{oc}", tag=f"bf_t{oc}")
                nc.sync.dma_start(out=t1, in_=bf[oc * 128 : (oc + 1) * 128, :])
                bf_t.append(t1)
                t2 = wts.tile([128, 1], F32, name=f"bg_t{oc}", tag=f"bg_t{oc}")
                nc.sync.dma_start(out=t2, in_=bg[oc * 128 : (oc + 1) * 128, :])
                bg_t.append(t2)
            bh_t = wts.tile([1, C], BF16, name="bh_t", tag="bh_t")
            nc.gpsimd.dma_start(out=bh_t, in_=bh[:, :])
            ones_t = wts.tile([1, 128], BF16, name="ones_t", tag="ones_t")
            nc.vector.memset(ones_t, 1.0)
            shift_t = wts.tile([128, 1], F32, name="shift_t", tag="shift_t")
            nc.vector.memset(shift_t, EXP_SHIFT)

            for b in range(NB):
                # ================= phase A =================
                # W input -> F (then X reuses the same slots -> G, HxT)
                w_t = []
                for kc in range(KC):
                    wt = xw.tile([128, N], F32R, name=f"w_{b}_{kc}", tag="xw")
                    for hh in range(2):
                        st = stg.tile([128, N // 2], F32, name="stg_t", tag="stg")
                        nc.sync.dma_start(
                            out=st,
                            in_=w[
                                b,
                                kc * 128 : (kc + 1) * 128,
                                hh * (N // 2) : (hh + 1) * (N // 2),
                            ],
                        )
                        nc.gpsimd.tensor_copy(
                            wt[:, hh * (N // 2) : (hh + 1) * (N // 2)], st
                        )
                    w_t.append(wt)
                F_t = [
                    fg.tile([128, N], F32R, name=f"F{b}{oc}", tag=f"F{oc}")
                    for oc in range(OC)
                ]
                G_t = [
                    fg.tile([128, N], F32R, name=f"G{b}{oc}", tag=f"G{oc}")
                    for oc in range(OC)
                ]
                for oc in range(OC):
                    for i0, iw in ISL:
                        p = ps.tile([128, 512], F32, name="psA", tag="out", bufs=5)
                        for kc in range(KC):
                            nc.tensor.matmul(
                                p[:, :iw],
                                wfT_t[kc][:, oc * 128 : (oc + 1) * 128],
                                w_t[kc][:, i0 : i0 + iw],
                                start=(kc == 0),
                                stop=(kc == KC - 1),
                            )
                        nc.vector.tensor_scalar_add(
                            F_t[oc][:, i0 : i0 + iw], p[:, :iw], bf_t[oc]
                        )
                x_t = []
                for kc in range(KC):
                    xt = xw.tile([128, N], F32R, name=f"x_{b}_{kc}", tag="xw")
                    for hh in range(2):
                        st = stg.tile([128, N // 2], F32, name="stg_t", tag="stg")
                        nc.sync.dma_start(
                            out=st,
                            in_=x[
                                b,
                                kc * 128 : (kc + 1) * 128,
                                hh * (N // 2) : (hh + 1) * (N // 2),
                            ],
                        )
                        nc.gpsimd.tensor_copy(
                            xt[:, hh * (N // 2) : (hh + 1) * (N // 2)], st
                        )
                    x_t.append(xt)
                for oc in range(OC):
                    for i0, iw in ISL:
                        p = ps.tile([128, 512], F32, name="psG", tag="out", bufs=5)
                        for kc in range(KC):
                            nc.tensor.matmul(
                                p[:, :iw],
                                wgT_t[kc][:, oc * 128 : (oc + 1) * 128],
                                x_t[kc][:, i0 : i0 + iw],
                                start=(kc == 0),
                                stop=(kc == KC - 1),
                            )
                        nc.vector.tensor_scalar_add(
                            G_t[oc][:, i0 : i0 + iw], p[:, :iw], bg_t[oc]
                        )
                hxTb = []
                for jb in range(NJB):
                    p = ps.tile([128, 512], F32, name="psH", tag="out", bufs=5)
                    for kc in range(KC):
                        nc.tensor.matmul(
                            p,
                            x_t[kc][:, jb * 128 : (jb + 1) * 128],
                            whT_t[kc],
                            start=(kc == 0),
                            stop=False,
                        )
                    nc.tensor.matmul(p, ones_t, bh_t, start=False, stop=True)
                    ht = hx.tile(
                        [128, C], BF16, name=f"hxT{b}_{jb}", tag=f"hxT{jb}"
                    )
                    nc.vector.tensor_copy(ht, p)
                    hxTb.append(ht)

                # ======== phases B/C in two j-halves ========
                acc_t = [
                    accp.tile([128, N], BF16, name=f"acc{b}{cc}", tag=f"acc{cc}")
                    for cc in range(CCH)
                ]
                for half in range(2):
                    jbs = range(half * JH, (half + 1) * JH)
                    eb_half = []
                    # ---- phase B: attention rows + softmax ----
                    for jb in jbs:
                        eb_t = ebp.tile([128, N], BF16, name=f"eb{jb}", tag="eb")
                        part = smalls.tile(
                            [128, 8], F32, name=f"part{jb}", tag="part"
                        )
                        for it, (i0, iw) in enumerate(ISL):
                            p = ps.tile(
                                [128, 512], F32, name="psAt", tag="attn", bufs=3
                            )
                            for oc in range(OC):
                                nc.tensor.matmul(
                                    p[:, :iw],
                                    G_t[oc][:, jb * 128 : (jb + 1) * 128],
                                    F_t[oc][:, i0 : i0 + iw],
                                    start=(oc == 0),
                                    stop=(oc == OC - 1),
                                )
                            nc.scalar.activation(
                                eb_t[:, i0 : i0 + iw],
                                p[:, :iw],
                                mybir.ActivationFunctionType.Exp,
                                bias=shift_t,
                                scale=1.0,
                                accum_out=part[:, it : it + 1],
                            )
                        rs = smalls.tile([128, 1], F32, name=f"rs{jb}", tag="rs")
                        nc.vector.reduce_sum(
                            out=rs, in_=part[:, 0:5], axis=mybir.AxisListType.X
                        )
                        rinv = smalls.tile(
                            [128, 1], F32, name=f"rinv{jb}", tag="rinv"
                        )
                        nc.vector.reciprocal(rinv, rs)
                        nc.vector.tensor_scalar_mul(eb_t, eb_t, rinv)
                        nc.sync.dma_start(
                            out=out2[b, jb * 128 : (jb + 1) * 128, :], in_=eb_t
                        )
                        eb_half.append(eb_t)
                    # ---- phase C: out += HxT.T @ Eb for this half ----
                    for i0, iw in ISL:
                        for cc in range(CCH):
                            p = ps.tile(
                                [128, 512], F32, name="psO", tag="out", bufs=5
                            )
                            for k, jb in enumerate(jbs):
                                nc.tensor.matmul(
                                    p[:, :iw],
                                    hxTb[jb][:, cc * 128 : (cc + 1) * 128],
                                    eb_half[k][:, i0 : i0 + iw],
                                    start=(k == 0),
                                    stop=(k == JH - 1),
                                )
                            if half == 0:
                                nc.vector.tensor_copy(
                                    acc_t[cc][:, i0 : i0 + iw], p[:, :iw]
                                )
                            else:
                                e = outp.tile(
                                    [128, 512], F32, name="e_t", tag="e_t", bufs=2
                                )
                                nc.vector.tensor_add(
                                    e[:, :iw], p[:, :iw], acc_t[cc][:, i0 : i0 + iw]
                                )
                                wct = outp.tile(
                                    [128, 512], F32, name="wct", tag="wct", bufs=2
                                )
                                nc.sync.dma_start(
                                    out=wct[:, :iw],
                                    in_=w[b, cc * 128 : (cc + 1) * 128, i0 : i0 + iw],
                                )
                                o = outp.tile(
                                    [128, 512], F32, name="o_t", tag="o_t", bufs=2
                                )
                                nc.scalar.mul(e[:, :iw], e[:, :iw], gamma)
                                nc.vector.tensor_add(o[:, :iw], e[:, :iw], wct[:, :iw])
                                nc.sync.dma_start(
                                    out=out1[
                                        b, cc * 128 : (cc + 1) * 128, i0 : i0 + iw
                                    ],
                                    in_=o[:, :iw],
                                )
    nc.finalize()
    return nc


_CACHE = {}


def kernel(x, w, Wf, bf, Wg, bg, Wh, bh, gamma):
    x = np.asarray(x, dtype=np.float32).reshape(B, C, N)
    w = np.asarray(w, dtype=np.float32).reshape(B, C, N)
    g = float(np.asarray(gamma))
    key = ("k", g)
    if key not in _CACHE:
        _CACHE[key] = build(g)
    nc = _CACHE[key]

    wfT = np.ascontiguousarray(np.asarray(Wf, np.float32).T)
    wgT = np.ascontiguousarray(np.asarray(Wg, np.float32).T)
    whT = np.ascontiguousarray(np.asarray(Wh, np.float32).T)
    bf2 = np.ascontiguousarray(np.asarray(bf, np.float32).reshape(C2, 1))
    bg2 = np.ascontiguousarray(np.asarray(bg, np.float32).reshape(C2, 1))
    bh2 = np.ascontiguousarray(np.asarray(bh, np.float32).reshape(1, C))

    in_maps = []
    for c in range(NCORES):
        in_maps.append(
            {
                "x": np.ascontiguousarray(x[c * NB : (c + 1) * NB]),
                "w": np.ascontiguousarray(w[c * NB : (c + 1) * NB]),
                "wfT": wfT,
                "wgT": wgT,
                "whT": whT,
                "bf": bf2,
                "bg": bg2,
                "bh": bh2,
            }
        )
    res = run_bass_kernel_spmd(nc, in_maps, core_ids=list(range(NCORES)))
    o1 = np.concatenate([r["out1"] for r in res.results], axis=0)
    o2 = np.concatenate(
        [r["out2"].astype(np.float32) for r in res.results], axis=0
    )
    return o1.reshape(B, C, H, W_), o2
